# revision 27
# baseline (speedup 1.0000x reference)
"""MetaSR super-resolution Trainium2 kernel.

Structure exploited: out_h=out_w=256 with H=W=64 LR grid means the scale
factor is exactly 4, so the nearest-neighbor gather index is iy=oy//4,
ix=ox//4 and the per-query MLP input collapses to 16 distinct subpixel
phases [dy/4, dx/4, 0.25].  The whole model becomes a 3x3 conv with 64
input / 48 output channels (3 RGB x 16 phases) + pixel shuffle, whose
48x576 weight predw = relu([16,3] @ w1 + b1) @ w2 + b2 is a tiny
16-phase MLP evaluated host-side (14 MFLOP of the model's 240 MFLOP;
the 226 MFLOP conv runs on device).

Sharding: data-parallel over LR rows (8 rows per core, 10-row halo band),
conv weights replicated.

The conv contraction (K = 9 taps x 64 ch = 576) is chunked K=128 by
pairing taps.  Each core holds the zero-padded band twice in a
128-partition tile at free-dim offsets that differ by the two taps'
shift delta, so one K=128 matmul consumes two taps without
materializing the unfolded tensor:
  band free index = r*66 + x  (66-wide zero-padded rows), tap (ki,kj)
  shift = ki*66 + kj; taps are paired with shift deltas 1 or 64.
Chunks alternate between two PSUM banks (summed at the end) so
successive matmuls never accumulate into the same bank back-to-back.

Band and weights are bf16 (PSUM accumulates fp32; measured rel err
~2.4e-3 vs the 2e-2 gate): halves DMA traffic.  The output is written
back as bf16 too and widened host-side.

All DMAs ride a single HWDGE queue (SP): both HWDGE queues share the
same 16 SDMA engines, so a second queue adds no bandwidth, but every
declared queue ring grows the runtime's fixed kernel postamble
(semaphore-clear sweep).  The unused SWDGE (qPoolDynamic) and ACT
queue declarations are stripped from the module before compile for the
same reason.

A run of dummy matmuls (zero scratch, overwritten by the first conv
accumulation via start=True) warms the PE HAM clock gate while the
DMAs land.
"""

import os

import ml_dtypes
import numpy as np

try:
    import concourse.bass as bass
except ImportError:  # fall back to the repo checkout
    import sys
    sys.path.insert(0, "/opt/trn_rl_repo")
    import concourse.bass as bass
import concourse.mybir as mybir
import concourse.tile as tile
from concourse import bacc
from concourse.bass_utils import run_bass_kernel_spmd

F32 = mybir.dt.float32
F32R = mybir.dt.float32r
BF16 = mybir.dt.bfloat16
N_CORES = 8
ROWS_PER_CORE = 8          # LR rows per core
BAND_ROWS = ROWS_PER_CORE + 2
NPOS = ROWS_PER_CORE * 64  # 512 LR positions per core

# Taps t = ki*3+kj have band shift ki*66+kj.  The band tile holds the
# zero-padded band twice: p0-63 = band@+1 (a chunk at rhs offset roff
# sees shift roff-1), p64-127 = band@0 (shift roff).  K=128 chunks pair
# the two taps with shift delta 1 in the partition dim; the three taps
# without a delta-1 partner run as K=64 chunks on one half.  Everything
# reads one tile -> a single input DMA.
#   (rhs_offset, K, taps, p_base)
ORDER = [
    (1, 128, (0, 1), 0),      # shifts 0,1
    (68, 128, (4, 5), 0),     # shifts 67,68
    (133, 128, (6, 7), 0),    # shifts 132,133
    (3, 64, (2,), 0),         # shift 2
    (67, 64, (3,), 0),        # shift 66
    (135, 64, (8,), 0),       # shift 134
]
COLS_B1 = 663  # 661 + pad cols so the tap-8 chunk's AP (135 + 8*66) fits
COLS_W = 6 * 48
COLS_A = COLS_B1 + COLS_W  # band ++ W in one blob -> one fat DMA

N_WARMUP_MM = 4

USE_BF16 = os.environ.get("METASR_DTYPE", "bf16") == "bf16"
QUEUE_MODE = os.environ.get("METASR_QUEUES", "sp16")

_CACHE = {}


def _build_program(use_bf16, queue_mode):
    """Build + compile the single-core Bass program (same for all cores)."""
    nc = bacc.Bacc("TRN2", target_bir_lowering=False, debug=False)

    dt = BF16 if use_bf16 else F32R
    odt = BF16 if use_bf16 else F32
    blob_a_d = nc.dram_tensor("blob_a", [128, COLS_A], dt, kind="ExternalInput")
    out48 = nc.dram_tensor("out48", [48, NPOS], odt, kind="ExternalOutput")

    single_q = queue_mode == "sp16"

    with tile.TileContext(nc) as tc:
        with (
            tc.tile_pool(name="blobs", bufs=1) as blobs,
            tc.tile_pool(name="work", bufs=1) as work,
            tc.tile_pool(name="opool", bufs=1) as opool,
            tc.tile_pool(name="ps_rgb", bufs=1, space="PSUM") as ps_rgb,
        ):
            blob_a = blobs.tile([128, COLS_A], dt, tag="blob_a")
            nc.sync.dma_start(blob_a[:, :], blob_a_d[:, :])
            band1 = blob_a[:, 0:COLS_B1]
            wtile = blob_a[:, COLS_B1:COLS_A]

            # PE warm-up during the DMA phase: conv chunk 0 uses start=True,
            # which resets PSUM, so these contribute nothing.
            rgb_ps = ps_rgb.tile([48, NPOS], F32, tag="rgb")
            warm = work.tile([128, NPOS], BF16, tag="warm")
            nc.vector.memset(warm[:, :], 0.0)
            for _ in range(N_WARMUP_MM):
                nc.tensor.matmul(
                    rgb_ps[:, :], warm[:, 0:48], warm[:, 0:NPOS],
                    start=True, stop=True,
                )

            for m, (roff, K, _taps, pb) in enumerate(ORDER):
                rhs = band1[pb:pb + K, roff:roff + 8 * 66].rearrange(
                    "p (r c) -> p r c", c=66
                )[:, :, 0:64]
                nc.tensor.matmul(
                    rgb_ps[:, :], wtile[pb:pb + K, m * 48:(m + 1) * 48], rhs,
                    start=(m == 0), stop=(m == len(ORDER) - 1),
                )

            # ---- write out: one full-width cast copy + one DMA ----
            out_sb = opool.tile([48, NPOS], odt, tag="out")
            nc.vector.tensor_copy(out_sb[:, :], rgb_ps[:, :])
            nc.sync.dma_start(out48[:, :], out_sb[:, :])

    if single_q:
        # Both HWDGE queues share the 16 SDMA engines, and the runtime's
        # fixed postamble (semaphore-clear sweep) scales with declared
        # queue rings — keep only the SP HWDGE queue actually used.
        used = {"qSPDynamicHW"}
        nc.m.queues = [q for q in nc.m.queues if q.name in used]

    nc.compile()
    return nc


def _round_f32r(x):
    """Round fp32 to the fp32r-representable set (bf16 hi + bf16 lo pair)."""
    hi = x.astype(ml_dtypes.bfloat16).astype(np.float32)
    lo = (x - hi).astype(ml_dtypes.bfloat16).astype(np.float32)
    return hi + lo


def _host_prep(feat, w1, b1, w2, b2, use_bf16):
    """Compute the 16-phase conv weights and pack per-core band blobs."""
    feat = np.ascontiguousarray(np.asarray(feat, dtype=np.float32))[0]  # [64,64,64]
    w1 = np.asarray(w1, dtype=np.float32)
    b1 = np.asarray(b1, dtype=np.float32)
    w2 = np.asarray(w2, dtype=np.float32)
    b2 = np.asarray(b2, dtype=np.float32)

    dydx = np.arange(16)
    mlpin = np.stack(
        [dydx // 4 / 4.0, dydx % 4 / 4.0, np.full(16, 0.25)], axis=1
    ).astype(np.float32)  # [16, 3]
    h = np.maximum(mlpin @ w1 + b1, 0.0).astype(np.float32)      # [16, 256]
    pw = (h @ w2 + b2).astype(np.float32).reshape(16, 64, 9, 3)  # [ph, c, t, o]

    wblob = np.zeros((128, COLS_W), dtype=np.float32)
    for m, (_roff, _K, taps, pb) in enumerate(ORDER):
        for slot, t in enumerate(taps):
            # rows pb + slot*64 + c ; cols m*48 + o*16 + ph
            r0 = pb + slot * 64
            wblob[r0:r0 + 64, m * 48:(m + 1) * 48] = \
                pw[:, :, t, :].transpose(1, 2, 0).reshape(64, 48)

    featp = np.zeros((64, 66, 66), dtype=np.float32)
    featp[:, 1:65, 1:65] = feat

    if use_bf16:
        wblob = wblob.astype(ml_dtypes.bfloat16)
        featp = featp.astype(ml_dtypes.bfloat16)
    else:
        wblob = _round_f32r(wblob)
        featp = _round_f32r(featp)
    ndt = featp.dtype

    blobs_a = []
    for core in range(N_CORES):
        r0 = core * ROWS_PER_CORE
        band = featp[:, r0:r0 + BAND_ROWS, :].reshape(64, BAND_ROWS * 66)
        ab = np.zeros((128, COLS_A), dtype=ndt)
        ab[0:64, 1:661] = band
        ab[64:128, 0:660] = band
        ab[:, COLS_B1:COLS_A] = wblob
        blobs_a.append(ab)
    return blobs_a


def _assemble(per_core_out48):
    """[8 x [48, 512]] -> [1, 3, 256, 256]."""
    full = np.stack([np.asarray(o, dtype=np.float32) for o in per_core_out48])
    full = full.reshape(8, 3, 4, 4, 8, 64)               # [core, o, dy, dx, r, x]
    rgb = full.transpose(1, 0, 4, 2, 5, 3).reshape(3, 256, 256)
    return np.ascontiguousarray(rgb)[None]


def get_program():
    key = ("nc", USE_BF16, QUEUE_MODE)
    if key not in _CACHE:
        _CACHE[key] = _build_program(USE_BF16, QUEUE_MODE)
    return _CACHE[key]


def run(feat, w1, b1, w2, b2, out_h, out_w, trace=False, **spmd_kwargs):
    assert int(out_h) == 256 and int(out_w) == 256
    nc = get_program()
    blobs_a = _host_prep(feat, w1, b1, w2, b2, USE_BF16)
    in_maps = [{"blob_a": blobs_a[core]} for core in range(N_CORES)]
    res = run_bass_kernel_spmd(
        nc, in_maps, core_ids=list(range(N_CORES)), trace=trace, **spmd_kwargs
    )
    out = _assemble([res.results[core]["out48"] for core in range(N_CORES)])
    return out, res


def kernel(feat, w1, b1, w2, b2, out_h, out_w):
    out, _ = run(feat, w1, b1, w2, b2, out_h, out_w, trace=False)
    return out
